# revision 1
# baseline (speedup 1.0000x reference)
"""Trainium2 Bass kernel for nn_DA_affinity_attention (gnn_message_passing).

Math (per batch b):
  coord_aff[n,m,t] = exp(-|q_coord[n,t] - kv_coord[m,t]|)
  for i in 0..1:
    q_  = q  @ Wq[i].T                  # [Nq, 32]
    kv_ = kv @ Wkv[i].T ; k, v = split  # [Nkv, 32] each
    # pos_enc only matters through aff.sum(-1):
    #   sum_c pos_enc[n,m,c] = sum_t coord_aff[n,m,t] * wds[i][t],
    #   wds[i][t] = sum_c Wdelta[i][c,t]
    s[n,m] = (sum_c exp(-|q_[n,c]-k[m,c]|) + sum_t wds[i][t]*coord_aff[n,m,t]) / 32
    attn   = softmax(s, axis=m)         # scores bounded (0, ~1.2] -> no max-sub needed
    q      = attn @ v
  out = q @ Wp.T + bp

Sharding: B*Nq = 1024 query rows -> 128 rows per core (8 cores). Each core owns the
full kv/kv_coord of its batch. Pure SPMD, no collectives.

Per-core layout: query rows on the 128 partitions, kv index m on the free dim.
|q-k| per channel is one fused DVE tensor_scalar (op0=subtract with per-partition q
scalar, op1=abs_max with 0) over a [128, 2048] k-row broadcast (DMA with step-0
partition dim from a DRAM scratch holding k_T). exp(-x) batched on ScalarE over
channel groups; channel reduction via DVE tree adds; softmax exp on ScalarE;
P transposed tile-wise on TensorE; attn @ [v | ones] on TensorE gives both the
unnormalized output and the softmax denominator.
"""

import sys
from contextlib import ExitStack

for _p in ("/opt/trn_rl_repo",):
    if _p not in sys.path:
        sys.path.insert(0, _p)

import numpy as np

import concourse.bacc as bacc
import concourse.bass as bass
import concourse.mybir as mybir
import concourse.tile as tile
from concourse.bass_utils import run_bass_kernel_spmd
from concourse.masks import make_identity

B, NQ, NKV = 2, 512, 2048
C = 32          # ERP_DIM
ICO = 64        # ICO_DIM
ITERS = 2
P = 128         # query rows per core
NCORES = 8
NT = NKV // P   # kv tiles of 128
GC = 4          # channels per exp-group
NG = C // GC    # groups per iteration
SCALE = 1.0 / C

F32 = mybir.dt.float32
F16 = mybir.dt.float16

# heavy-path dtype (a_buf / tree / scores accumulation)
HDT = F16
HNP = np.float16


def _bcast128(row_ap):
    """[1, F] DRAM AP -> [128, F] AP with partition step 0 (DMA source only)."""
    ap_dims = [[0, P]] + [list(d) for d in row_ap.ap[1:]]
    return bass.AP(tensor=row_ap.tensor, offset=row_ap.offset, ap=ap_dims)


def build_program(reps=1, mode='full'):
    nc = bacc.Bacc("TRN2", target_bir_lowering=False, debug=False)

    # ---- per-core DRAM I/O ----
    qT_d = nc.dram_tensor("qT", [C, P], F32, kind="ExternalInput")        # q shard ^T
    qc_d = nc.dram_tensor("qc", [P, 3], F32, kind="ExternalInput")        # q_coord shard
    kvT_d = nc.dram_tensor("kvT", [ICO, NKV], F32, kind="ExternalInput")  # kv batch ^T
    kvcT_d = nc.dram_tensor("kvcT", [3, NKV], F16, kind="ExternalInput")  # kv_coord ^T
    wqT_d = nc.dram_tensor("wqT", [C, ITERS * C], F32, kind="ExternalInput")
    wkvT_d = nc.dram_tensor("wkvT", [ICO, ITERS * 2 * C], F32, kind="ExternalInput")
    wpT_d = nc.dram_tensor("wpT", [C, C], F32, kind="ExternalInput")
    bpb_d = nc.dram_tensor("bpb", [P, C], F32, kind="ExternalInput")      # bp bcast
    wds_d = nc.dram_tensor("wds", [P, ITERS * 3], F32, kind="ExternalInput")
    y_d = nc.dram_tensor("y", [P, C], F32, kind="ExternalOutput")

    with tile.TileContext(nc) as tc, ExitStack() as ctx:
        consts = ctx.enter_context(tc.tile_pool(name="consts", bufs=1))
        small = ctx.enter_context(tc.tile_pool(name="small", bufs=2))
        kb_pool = ctx.enter_context(tc.tile_pool(name="kb", bufs=4))
        a_pool = ctx.enter_context(tc.tile_pool(name="abuf", bufs=2))
        tree_pool = ctx.enter_context(tc.tile_pool(name="tree", bufs=1))
        nb_pool = ctx.enter_context(tc.tile_pool(name="nb", bufs=2))
        psA = ctx.enter_context(tc.tile_pool(name="psA", bufs=2, space="PSUM"))
        psB = ctx.enter_context(tc.tile_pool(name="psB", bufs=2, space="PSUM"))
        psC = ctx.enter_context(tc.tile_pool(name="psC", bufs=2, space="PSUM"))
        dram = ctx.enter_context(tc.tile_pool(name="dram", bufs=1, space="DRAM"))

        if reps > 1:
            _loop_cm = tc.For_i(0, reps, 1)
            _loop_cm.__enter__()
        for _rep in range(1):
            # ---- load constants ----
            ident = consts.tile([P, P], F32, tag="ident")
            make_identity(nc, ident)

            kvT_t = consts.tile([P, NKV], F32, tag="kvT")
            kvT = kvT_t[0:ICO, :]
            nc.default_dma_engine.dma_start(out=kvT, in_=kvT_d.ap())
            qT_t = consts.tile([P, P], F32, tag="qT")
            qT = qT_t[0:C, :]
            nc.default_dma_engine.dma_start(out=qT, in_=qT_d.ap())
            qc = consts.tile([P, 3], F32, tag="qc")
            nc.default_dma_engine.dma_start(out=qc, in_=qc_d.ap())
            wqT_t = consts.tile([P, ITERS * C], F32, tag="wqT")
            wqT = wqT_t[0:C, :]
            nc.default_dma_engine.dma_start(out=wqT, in_=wqT_d.ap())
            wkvT_t = consts.tile([P, ITERS * 2 * C], F32, tag="wkvT")
            wkvT = wkvT_t[0:ICO, :]
            nc.default_dma_engine.dma_start(out=wkvT, in_=wkvT_d.ap())
            wpT_t = consts.tile([P, C], F32, tag="wpT")
            wpT = wpT_t[0:C, :]
            nc.default_dma_engine.dma_start(out=wpT, in_=wpT_d.ap())
            bpb = consts.tile([P, C], F32, tag="bpb")
            nc.default_dma_engine.dma_start(out=bpb, in_=bpb_d.ap())
            wds = consts.tile([P, ITERS * 3], F32, tag="wds")
            nc.default_dma_engine.dma_start(out=wds, in_=wds_d.ap())

            # ---- coord affinity planes CA[t] = exp(-|qc - kvc|), [128, NKV] each ----
            CA = consts.tile([P, 3 * NKV], F16, tag="CA")
            for t in range(3):
                kbc = kb_pool.tile([P, NKV], F16, tag="kb")
                nc.default_dma_engine.dma_start(out=kbc, in_=_bcast128(kvcT_d.ap()[t : t + 1, :]))
                nc.scalar.activation(
                    CA[:, t * NKV : (t + 1) * NKV],
                    kbc,
                    mybir.ActivationFunctionType.Abs,
                    bias=qc[:, t : t + 1],
                    scale=-1.0,
                )
            nc.scalar.activation(CA, CA, mybir.ActivationFunctionType.Exp, scale=-1.0)

            # ---- k/v projections for both iterations ----
            # k_T[i] = (Wkv[i][:C] @ kv.T) : [C, NKV]; v[i] = kv @ Wkv[i][C:].T : [NKV, C]
            kT_dram = dram.tile([ITERS * C, NKV], F16, tag="kTd")
            v1 = []  # [P, NT*(C+1)] per iter: 33-col blocks [v_tile | ones]
            for i in range(ITERS):
                kT16_t = consts.tile([P, NKV], F16, tag=f"kT16_{i}")
                kT16 = kT16_t[0:C, :]
                for j in range(NKV // 512):
                    kps = psB.tile([P, 512], F32, tag="mm")
                    nc.tensor.matmul(
                        kps[0:C, :],
                        wkvT[:, i * 2 * C : i * 2 * C + C],
                        kvT[:, j * 512 : (j + 1) * 512],
                        start=True,
                        stop=True,
                    )
                    nc.scalar.copy(kT16[:, j * 512 : (j + 1) * 512], kps[0:C, :])
                nc.default_dma_engine.dma_start(
                    out=kT_dram[i * C : (i + 1) * C, :], in_=kT16
                )

                vi = consts.tile([P, NT * (C + 1)], F32, tag=f"v1_{i}")
                nc.vector.memset(vi, 1.0)  # ones in col 32 of each 33-block
                for t in range(NT):
                    vps = psB.tile([P, C], F32, tag="mm")
                    nc.tensor.matmul(
                        vps,
                        kvT[:, t * P : (t + 1) * P],
                        wkvT[:, i * 2 * C + C : (i + 1) * 2 * C],
                        start=True,
                        stop=True,
                    )
                    nc.scalar.copy(vi[:, t * (C + 1) : t * (C + 1) + C], vps)
                v1.append(vi)

            # ---- q projection for iter 0 ----
            qp = psB.tile([P, C], F32, tag="mm")
            nc.tensor.matmul(qp, qT, wqT[:, 0:C], start=True, stop=True)
            q_cur = small.tile([P, C], F32, tag="q_cur")
            nc.scalar.copy(q_cur, qp)

            # ---- iterations ----
            for i in range(ITERS):
                scores = small.tile([P, NKV], HDT, tag="scores")
                if mode == 'skip_hot':
                    nc.vector.memset(scores, 0.03125)
                # hot loop: affinity accumulation over channel groups
                for g in range(NG if mode != 'skip_hot' else 0):
                    a_buf = a_pool.tile([P, GC * NKV], HDT, tag="abuf")
                    for ci in range(GC):
                        c = g * GC + ci
                        kb = kb_pool.tile([P, NKV], F16, tag="kb")
                        if mode != 'no_bcast':
                            nc.default_dma_engine.dma_start(
                                out=kb, in_=_bcast128(kT_dram[i * C + c : i * C + c + 1, :])
                            )
                        else:
                            nc.vector.memset(kb[:, 0:1], 0.5)
                        a_sl = a_buf[:, ci * NKV : (ci + 1) * NKV]
                        if mode == 'dma_only':
                            continue
                        if (5 * c) % 13 < 5:  # ~40% of channels via ScalarE
                            nc.scalar.activation(
                                a_sl, kb, mybir.ActivationFunctionType.Abs,
                                bias=q_cur[:, c : c + 1], scale=-1.0,
                            )
                        else:  # DVE: d = k - q; |d| = max(d, -d)
                            nc.vector.tensor_scalar(
                                out=a_sl, in0=kb,
                                scalar1=q_cur[:, c : c + 1], scalar2=None,
                                op0=mybir.AluOpType.subtract,
                            )
                            nb = nb_pool.tile([P, NKV], F16, tag="nb")
                            nc.vector.tensor_scalar(
                                out=nb, in0=a_sl, scalar1=-1.0, scalar2=None,
                                op0=mybir.AluOpType.mult,
                            )
                            nc.vector.tensor_tensor(
                                out=a_sl, in0=a_sl, in1=nb, op=mybir.AluOpType.max
                            )
                    nc.scalar.activation(
                        a_buf, a_buf, mybir.ActivationFunctionType.Exp, scale=-1.0
                    )
                    tr = tree_pool.tile([P, 2 * NKV], HDT, tag="tree")
                    nc.vector.tensor_tensor(
                        out=tr,
                        in0=a_buf[:, 0 : 2 * NKV],
                        in1=a_buf[:, 2 * NKV : 4 * NKV],
                        op=mybir.AluOpType.add,
                    )
                    nc.vector.tensor_tensor(
                        out=tr[:, 0:NKV],
                        in0=tr[:, 0:NKV],
                        in1=tr[:, NKV : 2 * NKV],
                        op=mybir.AluOpType.add,
                    )
                    if g == 0:
                        nc.vector.tensor_copy(out=scores, in_=tr[:, 0:NKV])
                    else:
                        nc.vector.tensor_tensor(
                            out=scores, in0=scores, in1=tr[:, 0:NKV],
                            op=mybir.AluOpType.add,
                        )
                # position term: scores += sum_t wds[i][t] * CA[t]
                pos = small.tile([P, NKV], HDT, tag="pos")
                tmp = small.tile([P, NKV], HDT, tag="ptmp")
                nc.vector.tensor_scalar(
                    out=pos, in0=CA[:, 0:NKV],
                    scalar1=wds[:, 3 * i : 3 * i + 1], scalar2=None,
                    op0=mybir.AluOpType.mult,
                )
                for t in (1, 2):
                    nc.vector.tensor_scalar(
                        out=tmp, in0=CA[:, t * NKV : (t + 1) * NKV],
                        scalar1=wds[:, 3 * i + t : 3 * i + t + 1], scalar2=None,
                        op0=mybir.AluOpType.mult,
                    )
                    nc.vector.tensor_tensor(
                        out=pos, in0=pos, in1=tmp, op=mybir.AluOpType.add
                    )
                nc.vector.tensor_tensor(
                    out=scores, in0=scores, in1=pos, op=mybir.AluOpType.add
                )
                # softmax numerator (scores bounded small -> no max subtraction)
                Pm = small.tile([P, NKV], F32, tag="Pm")
                nc.scalar.activation(
                    Pm, scores, mybir.ActivationFunctionType.Exp, scale=SCALE
                )
                # transpose P tile-wise: P_T[t] = Pm[:, t*128:(t+1)*128].T
                PT = small.tile([P, NKV], F32, tag="PT")
                for t in range(NT):
                    tp = psA.tile([P, P], F32, tag="tp")
                    nc.tensor.transpose(tp, Pm[:, t * P : (t + 1) * P], ident)
                    nc.scalar.copy(PT[:, t * P : (t + 1) * P], tp)
                # attn @ [v | ones] accumulated over kv tiles
                ap_ps = psC.tile([P, C + 1], F32, tag="attn")
                for t in range(NT):
                    nc.tensor.matmul(
                        ap_ps,
                        PT[:, t * P : (t + 1) * P],
                        v1[i][:, t * (C + 1) : (t + 1) * (C + 1)],
                        start=(t == 0),
                        stop=(t == NT - 1),
                    )
                recip = small.tile([P, 1], F32, tag="recip")
                nc.vector.reciprocal(recip, ap_ps[:, C : C + 1])
                q_next = small.tile([P, C], F32, tag="q_next")
                nc.vector.tensor_scalar(
                    out=q_next, in0=ap_ps[:, 0:C],
                    scalar1=recip[:, 0:1], scalar2=None,
                    op0=mybir.AluOpType.mult,
                )
                # project for next iteration / output
                tq = psB.tile([P, P], F32, tag="mm")
                nc.tensor.transpose(tq[0:C, :], q_next, ident)
                qnT_t = small.tile([P, P], F32, tag="qnT")
                qnT = qnT_t[0:C, :]
                nc.scalar.copy(qnT, tq[0:C, :])
                if i + 1 < ITERS:
                    qp2 = psB.tile([P, C], F32, tag="mm")
                    nc.tensor.matmul(
                        qp2, qnT, wqT[:, (i + 1) * C : (i + 2) * C], start=True, stop=True
                    )
                    q_cur = small.tile([P, C], F32, tag="q_cur")
                    nc.scalar.copy(q_cur, qp2)
                else:
                    fp = psB.tile([P, C], F32, tag="mm")
                    nc.tensor.matmul(fp, qnT, wpT, start=True, stop=True)
                    out_sb = small.tile([P, C], F32, tag="out_sb")
                    nc.vector.tensor_tensor(
                        out=out_sb, in0=fp, in1=bpb, op=mybir.AluOpType.add
                    )
                    nc.default_dma_engine.dma_start(out=y_d.ap(), in_=out_sb)

        if reps > 1:
            _loop_cm.__exit__(None, None, None)

    nc.compile()
    return nc


def make_in_maps(q, q_coord, kv, kv_coord, Wq, Wkv, Wdelta, Wp, bp):
    """Host-side sharding/layout prep. Core r handles batch r//4, rows (r%4)*128:."""
    q = np.asarray(q, np.float32)
    q_coord = np.asarray(q_coord, np.float32)
    kv = np.asarray(kv, np.float32)
    kv_coord = np.asarray(kv_coord, np.float32)
    Wq = np.asarray(Wq, np.float32)
    Wkv = np.asarray(Wkv, np.float32)
    Wdelta = np.asarray(Wdelta, np.float32)
    Wp = np.asarray(Wp, np.float32)
    bp = np.asarray(bp, np.float32)

    wqT = np.concatenate([Wq[i].T for i in range(ITERS)], axis=1).copy()      # [C, 2C]
    wkvT = np.concatenate([Wkv[i].T for i in range(ITERS)], axis=1).copy()    # [ICO, 2*2C]
    wpT = Wp.T.copy()                                                          # [C, C]
    bpb = np.broadcast_to(bp, (P, C)).copy()                                   # [P, C]
    wds = np.broadcast_to(
        Wdelta.sum(axis=1).reshape(ITERS * 3), (P, ITERS * 3)
    ).copy()                                                                   # [P, 6]

    in_maps = []
    for r in range(NCORES):
        b, j = divmod(r, NQ // P)
        rows = slice(j * P, (j + 1) * P)
        in_maps.append(
            {
                "qT": q[b, rows].T.copy(),
                "qc": q_coord[b, rows].copy(),
                "kvT": kv[b].T.copy(),
                "kvcT": kv_coord[b].T.astype(np.float16).copy(),
                "wqT": wqT,
                "wkvT": wkvT,
                "wpT": wpT,
                "bpb": bpb,
                "wds": wds,
            }
        )
    return in_maps


_PROGRAM = None


def kernel(q, q_coord, kv, kv_coord, Wq, Wkv, Wdelta, Wp, bp):
    global _PROGRAM
    if _PROGRAM is None:
        _PROGRAM = build_program()
    in_maps = make_in_maps(q, q_coord, kv, kv_coord, Wq, Wkv, Wdelta, Wp, bp)
    res = run_bass_kernel_spmd(_PROGRAM, in_maps, core_ids=list(range(NCORES)))
    out = np.empty((B, NQ, C), np.float32)
    for r in range(NCORES):
        b, j = divmod(r, NQ // P)
        out[b, j * P : (j + 1) * P, :] = res.results[r]["y"]
    return out

